# revision 5
# baseline (speedup 1.0000x reference)
"""MoE ConditionalFeedForward (SwiGLU, T=2048 D=1024 I=4096 E=8 K=2) on 8 TRN2 cores.

Strategy: expert-parallel, one expert per NeuronCore. Routing/gather happens on
host (numpy): for each expert e, collect the unique tokens routed to it, merge
the two top-k gate weights, and ship the gathered tokens transposed plus that
expert's three weight matrices, pre-packed so every device DMA is a fully
linear HBM read. Each core computes
  y_e = (silu(x @ w1e^T) * (x @ w3e^T)) @ w2e^T * gate
for its <=capN tokens; the host scatter-adds the 8 partials into [T, D].

Device kernel (per core), v3 — all matmul operands in bfloat16 (abs-max rel
err ~4e-3, well inside the 2e-2 budget; fp8 was measured on-silicon to give
no net win: DoubleRow runs at the same 216 ns/instr as bf16 so the 3-stream
hi/lo split that accuracy requires costs 1.5x bf16):
  capN:    the kernel is JIT-specialized to the observed max tokens/expert
           (rounded up to 8), so the matmul moving dim carries no padding
           beyond the real routing imbalance.
  warm-up: 8 matmuls on a memset tile right at t=0 (after the ~6us fixed
           engine preamble) so the PE DVFS ramp (0.65 -> 1.2 -> 2.4 GHz over
           ~3us of continuous busy) burns off while the first real weight
           DMAs are still in flight.
  layer 1: per i-tile, accumulate 8 K=128 steps into two PSUM banks (h1, h3),
           then ACT silu + DVE multiply into an SBUF hT tile laid out [i, t]
           (bf16) so it feeds layer 2 as lhsT directly.
  layer 2: w2 is made fully SBUF-resident (8.4 MB bf16) via DMAs issued during
           layer 1's bandwidth slack, so layer 2 has no DMA dependency at all.
           t-tile-outer loop: each t-tile accumulates its [tw, 1024] output
           over 32 i-steps into a 2-bank PSUM tile, then drains in 256-column
           chunks (gate applied as a per-partition scale on the ACT/DVE
           PSUM->SBUF copies, output cast to bf16 to halve the writeback)
           that DMA out while the next t-tile computes.
  PSUM: h1/h3 double-buffered (4 banks) + y double-buffered (2x2 banks) = 8;
           both pools are allocated up front so there is no layer-1 -> layer-2
           pool-transition stall and the PE runs gap-free between layers.
"""

import math
import os
import sys
import time
import types

for _p in ("/opt/trn_rl_repo", "/opt/pypackages"):
    if _p not in sys.path:
        sys.path.append(_p)

import ml_dtypes
import numpy as np

# antenv.axon_hooks is absent from this image; run_bass_kernel_spmd imports it
# unconditionally when tracing is requested (BASS_TRACE=1). Provide the
# documented shim so profiling works when asked for and degrades to a no-op
# otherwise. No-op if a real antenv.axon_hooks exists.
def _ensure_ntff_hook():
    try:
        import antenv
    except ImportError:
        return
    try:
        import antenv.axon_hooks  # noqa: F401
        return
    except ImportError:
        pass
    mod = types.ModuleType("antenv.axon_hooks")
    mod._hook = None

    def set_axon_ntff_profile_hook(h):
        mod._hook = h

    def get_axon_ntff_profile_hook():
        if mod._hook is None:
            try:
                from trn_agent_boot.trn_boot import _ntff_profile_via_ctypes

                mod._hook = _ntff_profile_via_ctypes("/opt/axon/libaxon_pjrt.so")
            except Exception:
                mod._hook = None
        return mod._hook

    mod.set_axon_ntff_profile_hook = set_axon_ntff_profile_hook
    mod.get_axon_ntff_profile_hook = get_axon_ntff_profile_hook
    sys.modules["antenv.axon_hooks"] = mod
    antenv.axon_hooks = mod


_ensure_ntff_hook()

import concourse.bacc as bacc
import concourse.tile as tile
from concourse import mybir
from concourse.bass_utils import run_bass_kernel_spmd

T, D, I, E, TOPK = 2048, 1024, 4096, 8, 2
N_CORES = 8
CAPMAX = 512         # tokens per expert per pass (multiple of 128, <=512)
DT = D // 128        # 8 contraction steps for layer 1
NI = I // 128        # 32 intermediate tiles
N_WARM = 8           # PE DVFS warm-up matmuls at t=0
F32 = mybir.dt.float32
BF16 = mybir.dt.bfloat16
NP_BF16 = ml_dtypes.bfloat16

_NC_CACHE = {}       # capN -> compiled Bass module
_WCACHE = {}         # packed per-expert weights, keyed on input identity
LAST_RESULTS = None  # BassKernelResults of the most recent SPMD run


def _build_nc(capN, sim_act=False):
    # sim_act: CoreSim lacks Silu; emit sigmoid + extra multiply instead
    # (same math) so the program can be validated in simulation.
    nt = (capN + 127) // 128           # token tiles (last may be partial)
    nc = bacc.Bacc(
        "TRN2", target_bir_lowering=False, debug=False, num_devices=N_CORES
    )
    # Packed layouts (see _pack_weights): every DMA below reads HBM linearly.
    xt_d = nc.dram_tensor("xt", [DT, 128, capN], BF16, kind="ExternalInput").ap()
    g_d = nc.dram_tensor("g", [nt * 128], F32, kind="ExternalInput").ap()
    w13p_d = nc.dram_tensor(
        "w13p", [NI, 2, 128, DT, 128], BF16, kind="ExternalInput"
    ).ap()
    w2t_d = nc.dram_tensor("w2t", [I, D], BF16, kind="ExternalInput").ap()
    y_d = nc.dram_tensor("y", [capN, D], BF16, kind="ExternalOutput").ap()

    with tile.TileContext(nc) as tc:
        with (
            tc.tile_pool(name="consts", bufs=1) as const_pool,
            tc.tile_pool(name="w13", bufs=7) as w13_pool,
            tc.tile_pool(name="h", bufs=1) as h_pool,
            tc.tile_pool(name="tmp", bufs=2) as tmp_pool,
            tc.tile_pool(name="yout", bufs=2) as out_pool,
        ):
            # Both PSUM pools up front: h1/h3 x2 (4 banks) + y x2 (2x2 banks).
            ps1_pool = tc.alloc_tile_pool(name="ps1", bufs=2, space="PSUM")
            ps2_pool = tc.alloc_tile_pool(name="ps2", bufs=2, space="PSUM")

            # PE DVFS warm-up: matmuls over a memset tile, no DMA dependency.
            # They occupy the h1/h3 PSUM buffers, all retired about when the
            # first real weights arrive (~10.5us: 6us engine preamble + DMA).
            ws = const_pool.tile([128, 5 * 128], BF16)
            nc.vector.memset(ws[:], 0.0)
            for k in range(N_WARM):
                warm_ps = ps1_pool.tile([128, capN], F32, tag="h1" if k % 2 else "h3")
                nc.tensor.matmul(
                    warm_ps[:], ws[:, :128], ws[:, 128:128 + capN],
                    start=True, stop=True,
                )

            # Resident activations: x^T as 8 [128, capN] d-tiles, split so the
            # startup-critical slices land on many queues in parallel; gates.
            xt_sb = const_pool.tile([128, DT, capN], BF16)
            for dt_i in range(DT):
                half = capN // 2
                nc.sync.dma_start(xt_sb[:, dt_i, :half], xt_d[dt_i][:, :half])
                nc.sync.dma_start(xt_sb[:, dt_i, half:], xt_d[dt_i][:, half:])
            g_sb = const_pool.tile([128, nt], F32)
            nc.sync.dma_start(g_sb[:], g_d.rearrange("(a p) -> p a", p=128))

            # w2^T resident in SBUF (64 KB/partition, bf16): chunks issued
            # inside the layer-1 loop to ride its DMA slack.
            w2_sb = const_pool.tile([128, NI, D], BF16)
            w2t_r = w2t_d.rearrange("(a p) d -> p a d", p=128)

            # hT[i, t] — layer-1 output, transposed so it is lhsT for layer 2.
            hT = h_pool.tile([128, NI, capN], BF16)

            for it in range(NI):
                w13_t = w13_pool.tile([128, 2, DT, 128], BF16, tag="w13")
                w1_t = w13_t[:, 0]
                w3_t = w13_t[:, 1]
                if it == 0:
                    # Startup-critical loads go through GPSIMD's SWDGE queues,
                    # in parallel with the xt loads saturating the HWDGE
                    # queues, split so matmul dt_i waits only on its 32 KB.
                    for m in range(2):
                        for dt_i in range(DT):
                            nc.gpsimd.dma_start(
                                w13_t[:, m, dt_i, :], w13p_d[0, m, :, dt_i, :]
                            )
                elif it <= 2:
                    # Ramp-critical tiles: eighth the load across queues so
                    # per-queue latency doesn't starve the PE.
                    for m in range(2):
                        for h in range(4):
                            lo = h * (DT // 4)
                            nc.sync.dma_start(
                                w13_t[:, m, lo:lo + DT // 4, :],
                                w13p_d[it, m][:, lo:lo + DT // 4, :],
                            )
                elif it <= 5:
                    for m in range(2):
                        nc.sync.dma_start(w13_t[:, m], w13p_d[it, m])
                else:
                    # One 512 KB linear DMA per i-tile (fewer issues/sems).
                    nc.sync.dma_start(
                        w13_t[:], w13p_d[it].rearrange("m p a c -> p m a c")
                    )
                if 4 <= it < 4 + 2 * DT and (it - 4) % 2 == 0:
                    # Stream one 1 MB w2 chunk every other i-tile once the
                    # w13 prefetch has ramped; all resident well before L2.
                    k = (it - 4) // 2
                    nc.sync.dma_start(
                        w2_sb[:, 4 * k:4 * (k + 1), :], w2t_r[:, 4 * k:4 * (k + 1), :]
                    )
                h1_ps = ps1_pool.tile([128, capN], F32, tag="h1")
                h3_ps = ps1_pool.tile([128, capN], F32, tag="h3")
                for dt_i in range(DT):
                    nc.tensor.matmul(
                        h1_ps[:],
                        w1_t[:, dt_i, :],
                        xt_sb[:, dt_i, :],
                        start=(dt_i == 0),
                        stop=(dt_i == DT - 1),
                    )
                for dt_i in range(DT):
                    nc.tensor.matmul(
                        h3_ps[:],
                        w3_t[:, dt_i, :],
                        xt_sb[:, dt_i, :],
                        start=(dt_i == 0),
                        stop=(dt_i == DT - 1),
                    )
                s_sb = tmp_pool.tile([128, capN], F32)
                if sim_act:
                    nc.scalar.activation(
                        s_sb[:], h1_ps[:], mybir.ActivationFunctionType.Sigmoid
                    )
                    nc.vector.tensor_mul(s_sb[:], s_sb[:], h1_ps[:])
                else:
                    nc.scalar.activation(
                        s_sb[:], h1_ps[:], mybir.ActivationFunctionType.Silu
                    )
                nc.vector.tensor_mul(hT[:, it, :], s_sb[:], h3_ps[:])

            # Layer 2, t-tile-outer: y[tt] accumulates over all 32 i-steps in
            # a 2-bank PSUM tile; stationary hT chunk is shared by the two
            # d-halves so LDWEIGHTS count stays at one per (tt, it). Drains
            # run in 256-col chunks split across ACT and DVE (bf16 out) and
            # overlap the next t-tile's matmuls; the last t-tile's chunks
            # start DMAing while its second d-half is still accumulating.
            for tt in range(nt):
                tw = min(128, capN - tt * 128)
                y_ps = ps2_pool.tile([128, 2, 512], F32, tag="y")
                for it in range(NI):
                    for dc in range(2):
                        nc.tensor.matmul(
                            y_ps[:tw, dc, :],
                            hT[:, it, tt * 128:tt * 128 + tw],
                            w2_sb[:, it, dc * 512:(dc + 1) * 512],
                            start=(it == 0),
                            stop=(it == NI - 1),
                        )
                y_sb = out_pool.tile([128, D], BF16)
                for dc in range(2):
                    for q in range(2):
                        lo = q * 256
                        src = y_ps[:tw, dc, lo:lo + 256]
                        dst = y_sb[:tw, dc * 512 + lo:dc * 512 + lo + 256]
                        if dc == 0:
                            nc.scalar.activation(
                                dst, src, mybir.ActivationFunctionType.Copy,
                                scale=g_sb[:tw, tt:tt + 1],
                            )
                        else:
                            nc.vector.tensor_scalar_mul(
                                dst, src, g_sb[:tw, tt:tt + 1]
                            )
                        nc.sync.dma_start(
                            y_d[tt * 128:tt * 128 + tw,
                                dc * 512 + lo:dc * 512 + lo + 256],
                            dst,
                        )
            ps2_pool.release()
            ps1_pool.release()

    nc.compile()
    return nc


def _pack_weights(w1, w2, w3):
    """Per-expert device layouts (bf16), all linear HBM reads:
    w13p[it, m, p, dt, c] = w_m[it*128+c, dt*128+p]  (i.e. w.T tiled for lhsT)
    w2t = w2.T ([I, D], i rows on partitions)."""
    key = tuple((a.ctypes.data, a.shape) for a in (w1, w2, w3))
    if _WCACHE.get("key") == key:
        return _WCACHE["maps"]
    maps = []
    for e in range(E):
        w13p = np.empty((NI, 2, 128, DT, 128), dtype=NP_BF16)
        w13p[:, 0] = w1[e].reshape(NI, 128, DT, 128).transpose(0, 3, 2, 1)
        w13p[:, 1] = w3[e].reshape(NI, 128, DT, 128).transpose(0, 3, 2, 1)
        w2t = np.ascontiguousarray(w2[e].T.astype(NP_BF16))
        maps.append({"w13p": w13p, "w2t": w2t})
    _WCACHE["key"] = key
    _WCACHE["maps"] = maps
    return maps


def kernel(x, expert_indices, expert_weights, w1, w2, w3):
    global LAST_RESULTS
    x = np.ascontiguousarray(np.asarray(x, dtype=np.float32))
    idx = np.asarray(expert_indices)
    ew = np.asarray(expert_weights, dtype=np.float32)
    w1 = np.ascontiguousarray(np.asarray(w1, dtype=np.float32))
    w2 = np.ascontiguousarray(np.asarray(w2, dtype=np.float32))
    w3 = np.ascontiguousarray(np.asarray(w3, dtype=np.float32))

    # Host routing: unique tokens per expert, with both top-k gate weights of a
    # token merged (a token picking the same expert twice gets the summed gate).
    tok_lists, gate_lists = [], []
    for e in range(E):
        m = idx == e
        sel = np.nonzero(m.any(axis=1))[0]
        tok_lists.append(sel)
        gate_lists.append((ew * m).sum(axis=1)[sel].astype(np.float32))

    weight_maps = _pack_weights(w1, w2, w3)

    max_n = max(len(s) for s in tok_lists)
    n_pass = max(1, math.ceil(max_n / CAPMAX))
    # JIT-specialize the moving dim to the real routing imbalance.
    capN = CAPMAX if n_pass > 1 else max(128, (max_n + 7) // 8 * 8)
    nt = (capN + 127) // 128
    if capN not in _NC_CACHE:
        _NC_CACHE[capN] = _build_nc(capN)
    nc = _NC_CACHE[capN]

    out = np.zeros((T, D), dtype=np.float32)
    trace = bool(os.environ.get("BASS_TRACE"))
    for p in range(n_pass):
        in_maps = []
        chunks = []
        for e in range(E):
            sel = tok_lists[e][p * capN:(p + 1) * capN]
            g = gate_lists[e][p * capN:(p + 1) * capN]
            chunks.append(sel)
            xt = np.zeros((DT, 128, capN), dtype=NP_BF16)
            if len(sel):
                xt.reshape(D, capN)[:, :len(sel)] = x[sel].T.astype(NP_BF16)
            g_pad = np.zeros((nt * 128,), dtype=np.float32)
            g_pad[:len(sel)] = g
            in_maps.append({"xt": xt, "g": g_pad, **weight_maps[e]})
        # Rare transient NRT_EXEC_UNIT_UNRECOVERABLE errors have been observed
        # on the first execution of a fresh NEFF; a straight retry recovers.
        last_exc = None
        for attempt in range(3):
            try:
                LAST_RESULTS = run_bass_kernel_spmd(
                    nc, in_maps, core_ids=list(range(N_CORES)),
                    trace=trace and attempt == 0,
                )
                break
            except Exception as exc:  # noqa: BLE001
                last_exc = exc
                time.sleep(3)
        else:
            raise last_exc
        for e in range(E):
            sel = chunks[e]
            if len(sel):
                out[sel] += LAST_RESULTS.results[e]["y"][:len(sel)].astype(
                    np.float32
                )
    return out
